# revision 43
# baseline (speedup 1.0000x reference)
"""Multi-head attention (B=8, N=1024, C=768, H=12, D=64) on 8 TRN2 NeuronCores.

Strategy: pure data-parallel over batch (B == n_cores == 8), no collectives.
Each core computes full 12-head attention for one batch element in a fully
transposed layout (channels on SBUF partitions).

v2 design (vs. the nb-serial baseline):
  - Heads are processed in PAIRS (2i, 2i+1).  The even head's K/Q live on
    SBUF partitions 0:64, the odd head's on 64:128, so the two QK^T matmuls
    (contraction 64 each) run CONCURRENTLY in the PE array via row tiling
    (tile_position (0,0) / (64,0)) writing different PSUM banks.
  - S-pair tiles are [128, 1024] f32 (2 PSUM banks): cols 0:512 even head,
    512:1024 odd head, for one (key-tile j, query-half nb).  One FD=1024
    ACT exp per tile halves the per-instruction overhead.
  - The additive attn bias is applied as exp(S/8)*exp(bias): exp(bias) is
    precomputed on host, loaded bf16, multiplied on DVE at 2x rate in SBUF
    (the f32-PSUM add of the baseline ran at 1x and cost 131us).
  - PV keeps the ones-column trick (out rows 0:64 = unnormalized out^T,
    row 64 = softmax sum).  The even head's two chains run inside the
    j-loop; the odd head's run as a dense 16-matmul block after them
    (PV holds 2 PSUM banks instead of 4, freeing 2 banks for a 'fill'
    tag so projection fillers don't serialize into the softmax S-tile
    rotation).  Each chain evacuates via one [65,512] DVE copy to SBUF,
    then gpsimd DMAs split rows 0:64 -> atb (partition-shifted for odd
    heads) and softmax-sum row 64 straight to DRAM scratch.
  - Normalization in 3 batches (heads 0-7, 8-9, 10-11) overlapped with
    attention; the last two heads' normalize-multiply runs on DVE to
    shorten the tail.  Whole norm DMA chain rides the sync queue (FIFO).
  - K/Q projection tiles for pair i+1 and redundant PE warm-keeper
    matmuls are interleaved into pair i's attention to keep the PE dense
    (HAM un-throttled).
  - Output projection: [128,512] tiles; ci 0..4 (heads 0..9) of the
    first 8 tiles accumulate BEFORE the heads-10/11 normalization so the
    PE works through the final norm round trip; stores on the idle sync
    queue.
"""

import os
import sys
import numpy as np

for _p in ("/opt/trn_rl_repo", "/root/.axon_site/_ro/trn_rl_repo"):
    if os.path.isdir(_p) and _p not in sys.path:
        sys.path.append(_p)

import ml_dtypes

BF16 = ml_dtypes.bfloat16

B, N, C = 8, 1024, 768
H, D = 12, 64
CT = C // 128         # 6 channel tiles
NT = N // 128         # 8 key tiles
F = 512
NP = H // 2           # 6 head pairs
HA = 8                # heads in normalization batch A (then 8-9, 10-11)

_cache = {}


def _build():
    import concourse.bass as bass
    import concourse.tile as tile
    from concourse import bacc, mybir

    f32 = mybir.dt.float32
    bf16 = mybir.dt.bfloat16
    AF = mybir.ActivationFunctionType
    ALU = mybir.AluOpType

    nc = bacc.Bacc("TRN2", target_bir_lowering=False)

    xT_d = nc.dram_tensor("xT", [C, N], bf16, kind="ExternalInput")
    wqT_d = nc.dram_tensor("wqT", [C, C], bf16, kind="ExternalInput")
    wkT_d = nc.dram_tensor("wkT", [C, C], bf16, kind="ExternalInput")
    wvT_d = nc.dram_tensor("wvT", [C, C], bf16, kind="ExternalInput")
    wpT_d = nc.dram_tensor("wpT", [C, C], bf16, kind="ExternalInput")
    bpT_d = nc.dram_tensor("bpT", [128, CT], f32, kind="ExternalInput")
    # exp(attn_bias) packed per (pair, key-tile j, query-half nb):
    # [...,0:512] = even head, [...,512:1024] = odd head
    eb_d = nc.dram_tensor("ebPk", [NP, NT, 2, 128, 2 * F], bf16,
                          kind="ExternalInput")
    outT_d = nc.dram_tensor("outT", [C, N], f32, kind="ExternalOutput")
    # softmax-sum scratch per normalization batch
    sA_scr = nc.dram_tensor("sA_scr", [HA * N], bf16)
    sB1_scr = nc.dram_tensor("sB1_scr", [2 * N], bf16)
    sB2_scr = nc.dram_tensor("sB2_scr", [2 * N], bf16)
    rA_scr = nc.dram_tensor("rA_scr", [1, HA * N], bf16)
    rB1_scr = nc.dram_tensor("rB1_scr", [1, 2 * N], bf16)
    rB2_scr = nc.dram_tensor("rB2_scr", [1, 2 * N], bf16)

    with tile.TileContext(nc) as tc:
        with tc.tile_pool(name="persist", bufs=1) as pers:
            xTb = pers.tile([128, CT, N], bf16, tag="xT")
            wqb = pers.tile([128, CT, C], bf16, tag="wq")
            wkb = pers.tile([128, CT, C], bf16, tag="wk")
            wvb = pers.tile([128, CT, C], bf16, tag="wv")
            wpb = pers.tile([128, CT, C], bf16, tag="wp")
            bpb = pers.tile([128, CT], f32, tag="bp")
            # row 64 collects softmax sums (same partition as pv row 64)
            rba = pers.tile([128, H * N], bf16, tag="rba")
            qtb = pers.tile([128, CT, N], bf16, tag="qt")
            ktb = pers.tile([128, CT, N], bf16, tag="kt")
            vb = pers.tile([128, NT, H, D + 1], bf16, tag="v")
            atb = pers.tile([128, CT, N], bf16, tag="at")

            for q0 in range(0, N, 256):
                nc.sync.dma_start(
                    xTb[:, :, q0:q0 + 256],
                    xT_d[:, q0:q0 + 256].rearrange(
                        "(ci p) n -> p ci n", p=128))
            nc.scalar.dma_start(
                wvb[:, :, 0:512],
                wvT_d[:, 0:512].rearrange("(ci p) o -> p ci o", p=128))
            nc.scalar.dma_start(
                wvb[:, :, 512:C],
                wvT_d[:, 512:C].rearrange("(ci p) o -> p ci o", p=128))
            nc.gpsimd.dma_start(
                wkb, wkT_d[:].rearrange("(ci p) o -> p ci o", p=128))
            nc.scalar.dma_start(
                wqb, wqT_d[:].rearrange("(ci p) o -> p ci o", p=128))
            nc.scalar.dma_start(bpb, bpT_d[:])

            nc.vector.memset(vb[:, :, :, D:D + 1], 1.0)

            with tc.tile_pool(name="ups", bufs=2, space="PSUM") as pU, \
                 tc.tile_pool(name="pvps", bufs=2, space="PSUM") as pPV, \
                 tc.tile_pool(name="fillps", bufs=2, space="PSUM") as pF, \
                 tc.tile_pool(name="ebb", bufs=6) as ebp, \
                 tc.tile_pool(name="vstagb", bufs=4) as vstagp, \
                 tc.tile_pool(name="nrmb", bufs=1) as nrm, \
                 tc.tile_pool(name="ptb", bufs=18) as ptp:

                eb_tiles = {}

                def eb_load(pr, j):
                    ebt = ebp.tile([128, 2, 2 * F], bf16, tag="eb")
                    nc.sync.dma_start(
                        ebt, eb_d[pr, j].rearrange("nb p q -> p nb q"))
                    eb_tiles[(pr, j)] = ebt

                def v_proj(h0, nh, nt):
                    """V projection for heads [h0, h0+nh) at key-tile nt."""
                    f0, fw = h0 * D, nh * D
                    ps = pF.tile([128, F], f32, tag="fill")
                    for ci in range(CT):
                        nc.tensor.matmul(
                            ps[:, :fw],
                            lhsT=xTb[:, ci, nt * 128:(nt + 1) * 128],
                            rhs=wvb[:, ci, f0:f0 + fw],
                            start=(ci == 0),
                            stop=(ci == CT - 1),
                        )
                    nc.vector.tensor_copy(
                        vb[:, nt, h0:h0 + nh, 0:D],
                        ps[:, :fw].rearrange("p (h d) -> p h d", d=D),
                    )

                def pe_warm():
                    """Redundant 6-matmul group (recomputes k-projection
                    tile 0 into a dead PSUM tile, never read).  Emitted in
                    filler-starved stretches so the PE's activity monitor
                    does not re-throttle the clock (K=4/8) on micro-idle.
                    Lives on the 'fill' tag, decoupled from the softmax
                    S-tile rotation."""
                    ps = pF.tile([128, F], f32, tag="fill", name="warm")
                    for ci in range(CT):
                        nc.tensor.matmul(
                            ps,
                            lhsT=wkb[:, ci, 0:128],
                            rhs=xTb[:, ci, 0:F],
                            start=(ci == 0),
                            stop=(ci == CT - 1),
                        )

                def kq_sub(which, cot, nb):
                    wb, dst = (wkb, ktb) if which == "k" else (wqb, qtb)
                    ps = pF.tile([128, F], f32, tag="fill")
                    for ci in range(CT):
                        nc.tensor.matmul(
                            ps,
                            lhsT=wb[:, ci, cot * 128:(cot + 1) * 128],
                            rhs=xTb[:, ci, nb * F:(nb + 1) * F],
                            start=(ci == 0),
                            stop=(ci == CT - 1),
                        )
                    nc.vector.tensor_copy(
                        dst[:, cot, nb * F:(nb + 1) * F], ps)

                def kq_ct(cot):
                    for which in ("k", "q"):
                        for nb in range(2):
                            kq_sub(which, cot, nb)

                def qk_pair(pr, j, nb):
                    """Row-tiled QK^T for both heads of the pair: even head
                    on PE rows 0:64, odd head on rows 64:128, concurrent."""
                    ksl = slice(j * 128, (j + 1) * 128)
                    qsl = slice(nb * F, (nb + 1) * F)
                    sp = pU.tile([128, 2 * F], f32, tag="ps",
                                 name=f"sp_{pr}_{j}_{nb}")
                    nc.tensor.matmul(
                        sp[:, 0:F],
                        lhsT=ktb[0:64, pr, ksl],
                        rhs=qtb[0:64, pr, qsl],
                        start=True, stop=True,
                    )
                    nc.tensor.matmul(
                        sp[:, F:2 * F],
                        lhsT=ktb[64:128, pr, ksl],
                        rhs=qtb[64:128, pr, qsl],
                        start=True, stop=True,
                    )
                    return sp

                def pv_evac(pr, hl, pv_nb):
                    """Evacuate one head's two PV chains; softmax-sum row
                    64 goes straight to the DRAM scratch its norm batch
                    reads."""
                    h = 2 * pr + hl
                    s_scr, hh = (
                        (sA_scr, h) if h < HA
                        else (sB1_scr, h - HA) if h < HA + 2
                        else (sB2_scr, h - HA - 2))
                    for nb in range(2):
                        qsl = slice(nb * F, (nb + 1) * F)
                        vstag = vstagp.tile([D + 1, F], bf16, tag="vstag")
                        nc.vector.tensor_copy(vstag, pv_nb[nb])
                        nc.gpsimd.dma_start(
                            atb[64 * hl:64 * hl + 64, pr, qsl],
                            vstag[0:D, :])
                        nc.gpsimd.dma_start(
                            s_scr[hh * N + nb * F:hh * N + (nb + 1) * F],
                            vstag[D:D + 1, :])

                def attn_pair(pr, fillers):
                    """Attention for heads (2*pr, 2*pr+1).

                    Emission order per j: exp/mul for j, then QK for j+1
                    (so the PE never head-of-line blocks on the softmax
                    chain), then the even head's PV matmuls for j, then
                    fillers.  The odd head's PV chains run as one dense
                    16-matmul block at the end of the pair on the pt tiles
                    buffered in SBUF - this halves the PSUM footprint of
                    PV (2 banks instead of 4), making room for the
                    decoupled 'fill' tag, and packs PE work into the pair
                    boundary.
                    """
                    pv0 = [pPV.tile([D + 1, F], f32, tag="pv",
                                    name=f"pv_{pr}_0_{nb}")
                           for nb in range(2)]
                    sps = [qk_pair(pr, 0, nb) for nb in range(2)]
                    pts_all = []
                    for j in range(NT):
                        ebt = eb_tiles.pop((pr, j))
                        pts = []
                        for nb in range(2):
                            pt = ptp.tile([128, 2 * F], bf16, tag="pt",
                                          name=f"pt_{pr}_{j}_{nb}")
                            nc.scalar.activation(
                                pt, sps[nb], AF.Exp, scale=0.125)
                            nc.vector.tensor_tensor(
                                pt, pt, ebt[:, nb, :], ALU.mult)
                            pts.append(pt)
                        pts_all.append(pts)
                        if j + 1 < NT:
                            sps = [qk_pair(pr, j + 1, nb) for nb in range(2)]
                        for nb in range(2):
                            nc.tensor.matmul(
                                pv0[nb],
                                lhsT=vb[:, j, 2 * pr, :],
                                rhs=pts[nb][:, 0:F],
                                start=(j == 0),
                                stop=(j == NT - 1),
                            )
                        for fn in fillers.get(j, ()):
                            fn()
                    pv_evac(pr, 0, pv0)
                    pv1 = [pPV.tile([D + 1, F], f32, tag="pv",
                                    name=f"pv_{pr}_1_{nb}")
                           for nb in range(2)]
                    for j in range(NT):
                        for nb in range(2):
                            nc.tensor.matmul(
                                pv1[nb],
                                lhsT=vb[:, j, 2 * pr + 1, :],
                                rhs=pts_all[j][nb][:, F:2 * F],
                                start=(j == 0),
                                stop=(j == NT - 1),
                            )
                    pv_evac(pr, 1, pv1)

                def norm_batch(batch):
                    """Batched reciprocal of softmax sums for a head range.

                    The sync-queue DMAs are FIFO-ordered among themselves;
                    the s_scr reload depends on the pair-evacuation DMAs
                    (gpsimd queue) via the DRAM tensor dependency.
                    """
                    h0, nh, s_scr, r_scr = [
                        (0, HA, sA_scr, rA_scr),
                        (HA, 2, sB1_scr, rB1_scr),
                        (HA + 2, 2, sB2_scr, rB2_scr),
                    ][batch]
                    cols = nh * N // 128
                    sb = nrm.tile([128, HA * N // 128], bf16, tag="sb")
                    nc.sync.dma_start(
                        sb[:, :cols],
                        s_scr[:].rearrange("(p f) -> p f", p=128))
                    rc32 = nrm.tile([128, HA * N // 128], f32, tag="rc32")
                    nc.vector.reciprocal(rc32[:, :cols], sb[:, :cols])
                    rcb = nrm.tile([128, HA * N // 128], bf16, tag="rcb")
                    nc.vector.tensor_copy(rcb[:, :cols], rc32[:, :cols])
                    nc.sync.dma_start(
                        r_scr[0, :].rearrange("(p f) -> p f", p=128),
                        rcb[:, :cols])
                    nc.sync.dma_start(
                        rba[:, h0 * N:(h0 + nh) * N],
                        r_scr[:].to_broadcast([128, nh * N]))

                def norm_mul(h, engine=None):
                    ct, po = h // 2, 64 * (h % 2)
                    sl = atb[po:po + 64, ct, :]
                    eng = engine or nc.gpsimd
                    eng.tensor_tensor(
                        sl, sl, rba[po:po + 64, h * N:(h + 1) * N], ALU.mult)

                # ---- emission schedule --------------------------------
                EB_DEPTH = 6
                for t in range(EB_DEPTH):
                    eb_load(0, t)
                nc.sync.dma_start(
                    wpb, wpT_d[:].rearrange("(ci p) o -> p ci o", p=128))
                # kq first: its weights (gpsimd queue) land ~2.5us before
                # wv (scalar), so the PE starts - and HAM warms - earlier
                kq_ct(0)
                v_proj(0, 8, 0)
                v_proj(0, 8, 1)
                kq_sub("k", 1, 0)
                kq_sub("k", 1, 1)

                # fillers per pair: rest of the V projection inside pair 0,
                # kq projection for pair+1, rolling EB prefetch, V
                # projection for heads 8:12 during pairs 2-3, norm batches
                # late, PE warm-keepers in the filler-starved pairs 4-5.
                for pr in range(NP):
                    fill = {j: [] for j in range(NT)}
                    for j in range(NT):
                        t = pr * NT + j + EB_DEPTH
                        if t < NP * NT:
                            fill[j].append(
                                lambda a=t // NT, b=t % NT: eb_load(a, b))
                    nxt = pr + 1
                    if nxt < NP:
                        if pr > 0:
                            fill[1].append(lambda n=nxt: kq_sub("k", n, 0))
                            fill[2].append(lambda n=nxt: kq_sub("k", n, 1))
                        fill[4].append(lambda n=nxt: kq_sub("q", n, 0))
                        fill[5].append(lambda n=nxt: kq_sub("q", n, 1))
                    if pr == 0:
                        for nt in range(2, NT):
                            fill[nt - 2].append(lambda t=nt: v_proj(0, 8, t))
                    if pr == 2:
                        for nt in range(4):
                            fill[6].append(lambda t=nt: v_proj(8, 4, t))
                    if pr == 3:
                        for nt in range(4, NT):
                            fill[6].append(lambda t=nt: v_proj(8, 4, t))
                    if pr == 4:
                        # heads 0..7 sums are complete after pair 3 evac
                        fill[0].append(lambda: norm_batch(0))
                        for h in range(4):
                            fill[6].append(lambda hh=h: norm_mul(hh))
                        fill[3].append(pe_warm)
                        fill[7].append(pe_warm)
                    if pr == 5:
                        fill[0].append(lambda: norm_batch(1))
                        for h in range(4, 8):
                            fill[3].append(lambda hh=h: norm_mul(hh))
                        for h in range(8, 10):
                            fill[5].append(lambda hh=h: norm_mul(hh))
                        for j in (1, 3, 5, 7):
                            fill[j].append(pe_warm)
                    attn_pair(pr, fill)

                norm_batch(2)

            # ---- output projection ------------------------------------
            # [128, 512] granularity (one PSUM bank per tile, 8 in
            # flight).  Pass 1 (ci 0..4: heads 0..9, already normalized)
            # for the first 8 tiles is emitted BEFORE the heads-10/11
            # normalize-multiplies so the PE stays busy through the final
            # normalization round trip; each tile then finishes with the
            # ci=5 matmul, a small ACT bias-add, and a store on the (idle)
            # sync queue.  Emitting more than 8 pass-1 tiles would
            # deadlock the PE FIFO on pool-slot reuse.
            with tc.tile_pool(name="ops", bufs=8, space="PSUM") as pC, \
                 tc.tile_pool(name="otb", bufs=6) as otp:

                def oproj_acc(cot, nb, cis):
                    ps = pC.tile([128, F], f32, tag="o",
                                 name=f"po_{cot}_{nb}")
                    for i, ci in enumerate(cis):
                        nc.tensor.matmul(
                            ps,
                            lhsT=wpb[:, ci, cot * 128:(cot + 1) * 128],
                            rhs=atb[:, ci, nb * F:(nb + 1) * F],
                            start=(i == 0),
                            stop=False,
                        )
                    return ps

                def oproj_fin(ps, cot, nb):
                    nc.tensor.matmul(
                        ps,
                        lhsT=wpb[:, CT - 1, cot * 128:(cot + 1) * 128],
                        rhs=atb[:, CT - 1, nb * F:(nb + 1) * F],
                        start=False, stop=True,
                    )
                    ot = otp.tile([128, F], f32, tag="ot",
                                  name=f"ot_{cot}_{nb}")
                    nc.scalar.activation(
                        ot, ps, AF.Identity, bias=bpb[:, cot:cot + 1])
                    nc.sync.dma_start(
                        outT_d[cot * 128:(cot + 1) * 128,
                               nb * F:(nb + 1) * F], ot)

                pss = {}
                for cot in range(4):
                    for nb in range(2):
                        pss[(cot, nb)] = oproj_acc(cot, nb, range(CT - 1))
                norm_mul(10, nc.vector)
                norm_mul(11, nc.vector)
                for cot in range(4):
                    for nb in range(2):
                        oproj_fin(pss[(cot, nb)], cot, nb)
                for cot in range(4, CT):
                    for nb in range(2):
                        ps = oproj_acc(cot, nb, range(CT - 1))
                        oproj_fin(ps, cot, nb)

    nc.compile()
    return nc


def _get_nc():
    if "nc" not in _cache:
        _cache["nc"] = _build()
    return _cache["nc"]


def prep_in_maps(x, attn_bias, Wq, Wk, Wv, Wp, bp):
    """Host-side sharding + layout prep (transposes/casts/exp of bias)."""
    wqT = np.ascontiguousarray(Wq.T).astype(BF16)
    wkT = np.ascontiguousarray(Wk.T).astype(BF16)
    wvT = np.ascontiguousarray(Wv.T).astype(BF16)
    wpT = np.ascontiguousarray(Wp.T).astype(BF16)
    bpT = np.ascontiguousarray(bp.astype(np.float32).reshape(CT, 128).T)
    # exp(bias)^T packed per (pair, key-tile, query-half): see kernel docstr
    E = np.exp(attn_bias[0].astype(np.float32)).transpose(0, 2, 1)
    E = np.ascontiguousarray(E).reshape(H, NT, 128, 2, F)
    ebPk = np.empty((NP, NT, 2, 128, 2 * F), dtype=np.float32)
    for pr in range(NP):
        ebPk[pr, :, :, :, 0:F] = E[2 * pr].transpose(0, 2, 1, 3)
        ebPk[pr, :, :, :, F:2 * F] = E[2 * pr + 1].transpose(0, 2, 1, 3)
    ebPk = ebPk.astype(BF16)
    in_maps = []
    for b in range(B):
        in_maps.append({
            "xT": np.ascontiguousarray(x[b].T).astype(BF16),
            "wqT": wqT, "wkT": wkT, "wvT": wvT, "wpT": wpT,
            "bpT": bpT, "ebPk": ebPk,
        })
    return in_maps


def run(in_maps, trace=False, **kw):
    from concourse.bass_utils import run_bass_kernel_spmd

    nc = _get_nc()
    return run_bass_kernel_spmd(
        nc, in_maps, core_ids=list(range(B)), trace=trace, **kw
    )


def kernel(x, attn_bias, Wq, Wk, Wv, Wp, bp):
    res = run(prep_in_maps(x, attn_bias, Wq, Wk, Wv, Wp, bp))
    out = np.stack(
        [res.results[b]["outT"].T for b in range(B)]
    ).astype(np.float32)
    return out


# revision 44
# speedup vs baseline: 1.0136x; 1.0136x over previous
"""Multi-head attention (B=8, N=1024, C=768, H=12, D=64) on 8 TRN2 NeuronCores.

Strategy: pure data-parallel over batch (B == n_cores == 8), no collectives.
Each core computes full 12-head attention for one batch element in a fully
transposed layout (channels on SBUF partitions).

v2 design (vs. the nb-serial baseline):
  - Heads are processed in PAIRS (2i, 2i+1).  The even head's K/Q live on
    SBUF partitions 0:64, the odd head's on 64:128, so the two QK^T matmuls
    (contraction 64 each) run CONCURRENTLY in the PE array via row tiling
    (tile_position (0,0) / (64,0)) writing different PSUM banks.
  - S-pair tiles are [128, 1024] f32 (2 PSUM banks): cols 0:512 even head,
    512:1024 odd head, for one (key-tile j, query-half nb).  One FD=1024
    ACT exp per tile halves the per-instruction overhead.
  - The additive attn bias is applied as exp(S/8)*exp(bias): exp(bias) is
    precomputed on host, loaded bf16, multiplied on DVE at 2x rate in SBUF
    (the f32-PSUM add of the baseline ran at 1x and cost 131us).
  - PV keeps the ones-column trick (out rows 0:64 = unnormalized out^T,
    row 64 = softmax sum).  All four (head, nb) chains evacuate via one
    [65,512] DVE copy to SBUF, then gpsimd DMAs split rows 0:64 -> atb
    (partition-shifted for odd heads) and row 64 -> s_stage.
  - Normalization in 3 batches (heads 0-7, 8-9, 10-11) overlapped with
    attention; the last two heads' normalize-multiply runs on DVE to
    shorten the tail.  Whole norm DMA chain rides the sync queue (FIFO).
  - K/Q projection tiles for pair i+1 are interleaved into pair i's
    attention to keep the PE dense (HAM un-throttled).
"""

import os
import sys
import numpy as np

for _p in ("/opt/trn_rl_repo", "/root/.axon_site/_ro/trn_rl_repo"):
    if os.path.isdir(_p) and _p not in sys.path:
        sys.path.append(_p)

import ml_dtypes

BF16 = ml_dtypes.bfloat16

B, N, C = 8, 1024, 768
H, D = 12, 64
CT = C // 128         # 6 channel tiles
NT = N // 128         # 8 key tiles
F = 512
NP = H // 2           # 6 head pairs
HA = 8                # heads in normalization batch A (then 8-9, 10-11)

_cache = {}


def _build():
    import concourse.bass as bass
    import concourse.tile as tile
    from concourse import bacc, mybir

    f32 = mybir.dt.float32
    bf16 = mybir.dt.bfloat16
    AF = mybir.ActivationFunctionType
    ALU = mybir.AluOpType

    nc = bacc.Bacc("TRN2", target_bir_lowering=False)

    xT_d = nc.dram_tensor("xT", [C, N], bf16, kind="ExternalInput")
    wqT_d = nc.dram_tensor("wqT", [C, C], bf16, kind="ExternalInput")
    wkT_d = nc.dram_tensor("wkT", [C, C], bf16, kind="ExternalInput")
    wvT_d = nc.dram_tensor("wvT", [C, C], bf16, kind="ExternalInput")
    wpT_d = nc.dram_tensor("wpT", [C, C], bf16, kind="ExternalInput")
    bpT_d = nc.dram_tensor("bpT", [128, CT], f32, kind="ExternalInput")
    # exp(attn_bias) packed per (pair, key-tile j, query-half nb):
    # [...,0:512] = even head, [...,512:1024] = odd head
    eb_d = nc.dram_tensor("ebPk", [NP, NT, 2, 128, 2 * F], bf16,
                          kind="ExternalInput")
    outT_d = nc.dram_tensor("outT", [C, N], f32, kind="ExternalOutput")
    # softmax-sum scratch per normalization batch
    sA_scr = nc.dram_tensor("sA_scr", [HA * N], bf16)
    sB1_scr = nc.dram_tensor("sB1_scr", [2 * N], bf16)
    sB2_scr = nc.dram_tensor("sB2_scr", [2 * N], bf16)
    rA_scr = nc.dram_tensor("rA_scr", [1, HA * N], bf16)
    rB1_scr = nc.dram_tensor("rB1_scr", [1, 2 * N], bf16)
    rB2_scr = nc.dram_tensor("rB2_scr", [1, 2 * N], bf16)

    with tile.TileContext(nc) as tc:
        with tc.tile_pool(name="persist", bufs=1) as pers:
            xTb = pers.tile([128, CT, N], bf16, tag="xT")
            wqb = pers.tile([128, CT, C], bf16, tag="wq")
            wkb = pers.tile([128, CT, C], bf16, tag="wk")
            wvb = pers.tile([128, CT, C], bf16, tag="wv")
            wpb = pers.tile([128, CT, C], bf16, tag="wp")
            bpb = pers.tile([128, CT], f32, tag="bp")
            # row 64 collects softmax sums (same partition as pv row 64)
            rba = pers.tile([128, H * N], bf16, tag="rba")
            qtb = pers.tile([128, CT, N], bf16, tag="qt")
            ktb = pers.tile([128, CT, N], bf16, tag="kt")
            vb = pers.tile([128, NT, H, D + 1], bf16, tag="v")
            atb = pers.tile([128, CT, N], bf16, tag="at")

            for q0 in range(0, N, 256):
                nc.sync.dma_start(
                    xTb[:, :, q0:q0 + 256],
                    xT_d[:, q0:q0 + 256].rearrange(
                        "(ci p) n -> p ci n", p=128))
            nc.scalar.dma_start(
                wvb[:, :, 0:512],
                wvT_d[:, 0:512].rearrange("(ci p) o -> p ci o", p=128))
            nc.scalar.dma_start(
                wvb[:, :, 512:C],
                wvT_d[:, 512:C].rearrange("(ci p) o -> p ci o", p=128))
            nc.gpsimd.dma_start(
                wkb, wkT_d[:].rearrange("(ci p) o -> p ci o", p=128))
            nc.scalar.dma_start(
                wqb, wqT_d[:].rearrange("(ci p) o -> p ci o", p=128))
            nc.scalar.dma_start(bpb, bpT_d[:])

            nc.vector.memset(vb[:, :, :, D:D + 1], 1.0)

            with tc.tile_pool(name="ups", bufs=2, space="PSUM") as pU, \
                 tc.tile_pool(name="pvps", bufs=2, space="PSUM") as pPV, \
                 tc.tile_pool(name="fillps", bufs=2, space="PSUM") as pF, \
                 tc.tile_pool(name="ebb", bufs=4) as ebp, \
                 tc.tile_pool(name="vstagb", bufs=4) as vstagp, \
                 tc.tile_pool(name="nrmb", bufs=1) as nrm, \
                 tc.tile_pool(name="ptb", bufs=18) as ptp:

                eb_tiles = {}

                def eb_load(pr, j):
                    ebt = ebp.tile([128, 2, 2 * F], bf16, tag="eb")
                    nc.sync.dma_start(
                        ebt, eb_d[pr, j].rearrange("nb p q -> p nb q"))
                    eb_tiles[(pr, j)] = ebt

                def v_proj(h0, nh, nt):
                    """V projection for heads [h0, h0+nh) at key-tile nt."""
                    f0, fw = h0 * D, nh * D
                    ps = pF.tile([128, F], f32, tag="fill")
                    for ci in range(CT):
                        nc.tensor.matmul(
                            ps[:, :fw],
                            lhsT=xTb[:, ci, nt * 128:(nt + 1) * 128],
                            rhs=wvb[:, ci, f0:f0 + fw],
                            start=(ci == 0),
                            stop=(ci == CT - 1),
                        )
                    nc.vector.tensor_copy(
                        vb[:, nt, h0:h0 + nh, 0:D],
                        ps[:, :fw].rearrange("p (h d) -> p h d", d=D),
                    )

                def pe_warm():
                    """Redundant 6-matmul group (recomputes k-projection
                    tile 0 into a dead PSUM tile, never read).  Emitted in
                    filler-starved stretches so the PE's activity monitor
                    does not re-throttle the clock (K=4/8) on micro-idle.
                    Lives on the 'fill' tag, decoupled from the softmax
                    S-tile rotation."""
                    ps = pF.tile([128, F], f32, tag="fill", name="warm")
                    for ci in range(CT):
                        nc.tensor.matmul(
                            ps,
                            lhsT=wkb[:, ci, 0:128],
                            rhs=xTb[:, ci, 0:F],
                            start=(ci == 0),
                            stop=(ci == CT - 1),
                        )

                def kq_sub(which, cot, nb):
                    wb, dst = (wkb, ktb) if which == "k" else (wqb, qtb)
                    ps = pF.tile([128, F], f32, tag="fill")
                    for ci in range(CT):
                        nc.tensor.matmul(
                            ps,
                            lhsT=wb[:, ci, cot * 128:(cot + 1) * 128],
                            rhs=xTb[:, ci, nb * F:(nb + 1) * F],
                            start=(ci == 0),
                            stop=(ci == CT - 1),
                        )
                    nc.vector.tensor_copy(
                        dst[:, cot, nb * F:(nb + 1) * F], ps)

                def kq_ct(cot):
                    for which in ("k", "q"):
                        for nb in range(2):
                            kq_sub(which, cot, nb)

                def qk_pair(pr, j, nb):
                    """Row-tiled QK^T for both heads of the pair: even head
                    on PE rows 0:64, odd head on rows 64:128, concurrent."""
                    ksl = slice(j * 128, (j + 1) * 128)
                    qsl = slice(nb * F, (nb + 1) * F)
                    sp = pU.tile([128, 2 * F], f32, tag="ps",
                                 name=f"sp_{pr}_{j}_{nb}")
                    nc.tensor.matmul(
                        sp[:, 0:F],
                        lhsT=ktb[0:64, pr, ksl],
                        rhs=qtb[0:64, pr, qsl],
                        start=True, stop=True,
                    )
                    nc.tensor.matmul(
                        sp[:, F:2 * F],
                        lhsT=ktb[64:128, pr, ksl],
                        rhs=qtb[64:128, pr, qsl],
                        start=True, stop=True,
                    )
                    return sp

                def pv_evac(pr, hl, pv_nb):
                    """Evacuate one head's two PV chains; softmax-sum row
                    64 goes straight to the DRAM scratch its norm batch
                    reads."""
                    h = 2 * pr + hl
                    s_scr, hh = (
                        (sA_scr, h) if h < HA
                        else (sB1_scr, h - HA) if h < HA + 2
                        else (sB2_scr, h - HA - 2))
                    for nb in range(2):
                        qsl = slice(nb * F, (nb + 1) * F)
                        vstag = vstagp.tile([D + 1, F], bf16, tag="vstag")
                        nc.vector.tensor_copy(vstag, pv_nb[nb])
                        nc.gpsimd.dma_start(
                            atb[64 * hl:64 * hl + 64, pr, qsl],
                            vstag[0:D, :])
                        nc.gpsimd.dma_start(
                            s_scr[hh * N + nb * F:hh * N + (nb + 1) * F],
                            vstag[D:D + 1, :])

                def attn_pair(pr, fillers):
                    """Attention for heads (2*pr, 2*pr+1).

                    Emission order per j: exp/mul for j, then QK for j+1
                    (so the PE never head-of-line blocks on the softmax
                    chain), then the even head's PV matmuls for j, then
                    fillers.  The odd head's PV chains run as one dense
                    16-matmul block at the end of the pair on the pt tiles
                    buffered in SBUF - this halves the PSUM footprint of
                    PV (2 banks instead of 4), making room for the
                    decoupled 'fill' tag, and packs PE work into the pair
                    boundary.
                    """
                    pv0 = [pPV.tile([D + 1, F], f32, tag="pv",
                                    name=f"pv_{pr}_0_{nb}")
                           for nb in range(2)]
                    sps = [qk_pair(pr, 0, nb) for nb in range(2)]
                    pts_all = []
                    for j in range(NT):
                        ebt = eb_tiles.pop((pr, j))
                        pts = []
                        for nb in range(2):
                            pt = ptp.tile([128, 2 * F], bf16, tag="pt",
                                          name=f"pt_{pr}_{j}_{nb}")
                            nc.scalar.activation(
                                pt, sps[nb], AF.Exp, scale=0.125)
                            nc.vector.tensor_tensor(
                                pt, pt, ebt[:, nb, :], ALU.mult)
                            pts.append(pt)
                        pts_all.append(pts)
                        if j + 1 < NT:
                            sps = [qk_pair(pr, j + 1, nb) for nb in range(2)]
                        for nb in range(2):
                            nc.tensor.matmul(
                                pv0[nb],
                                lhsT=vb[:, j, 2 * pr, :],
                                rhs=pts[nb][:, 0:F],
                                start=(j == 0),
                                stop=(j == NT - 1),
                            )
                        for fn in fillers.get(j, ()):
                            fn()
                    pv_evac(pr, 0, pv0)
                    pv1 = [pPV.tile([D + 1, F], f32, tag="pv",
                                    name=f"pv_{pr}_1_{nb}")
                           for nb in range(2)]
                    for j in range(NT):
                        for nb in range(2):
                            nc.tensor.matmul(
                                pv1[nb],
                                lhsT=vb[:, j, 2 * pr + 1, :],
                                rhs=pts_all[j][nb][:, F:2 * F],
                                start=(j == 0),
                                stop=(j == NT - 1),
                            )
                    pv_evac(pr, 1, pv1)

                def norm_batch(batch):
                    """Batched reciprocal of softmax sums for a head range.

                    The sync-queue DMAs are FIFO-ordered among themselves;
                    the s_scr reload depends on the pair-evacuation DMAs
                    (gpsimd queue) via the DRAM tensor dependency.
                    """
                    h0, nh, s_scr, r_scr = [
                        (0, HA, sA_scr, rA_scr),
                        (HA, 2, sB1_scr, rB1_scr),
                        (HA + 2, 2, sB2_scr, rB2_scr),
                    ][batch]
                    cols = nh * N // 128
                    sb = nrm.tile([128, HA * N // 128], bf16, tag="sb")
                    nc.sync.dma_start(
                        sb[:, :cols],
                        s_scr[:].rearrange("(p f) -> p f", p=128))
                    rc32 = nrm.tile([128, HA * N // 128], f32, tag="rc32")
                    nc.vector.reciprocal(rc32[:, :cols], sb[:, :cols])
                    rcb = nrm.tile([128, HA * N // 128], bf16, tag="rcb")
                    nc.vector.tensor_copy(rcb[:, :cols], rc32[:, :cols])
                    nc.sync.dma_start(
                        r_scr[0, :].rearrange("(p f) -> p f", p=128),
                        rcb[:, :cols])
                    nc.sync.dma_start(
                        rba[:, h0 * N:(h0 + nh) * N],
                        r_scr[:].to_broadcast([128, nh * N]))

                def norm_mul(h, engine=None):
                    ct, po = h // 2, 64 * (h % 2)
                    sl = atb[po:po + 64, ct, :]
                    eng = engine or nc.gpsimd
                    eng.tensor_tensor(
                        sl, sl, rba[po:po + 64, h * N:(h + 1) * N], ALU.mult)

                # ---- emission schedule --------------------------------
                EB_DEPTH = 4
                for t in range(EB_DEPTH):
                    eb_load(0, t)
                nc.sync.dma_start(
                    wpb, wpT_d[:].rearrange("(ci p) o -> p ci o", p=128))
                v_proj(0, 8, 0)
                v_proj(0, 8, 1)
                kq_ct(0)
                kq_sub("k", 1, 0)
                kq_sub("k", 1, 1)

                # fillers per pair: rest of the V projection inside pair 0,
                # kq projection for pair+1, rolling EB prefetch, V
                # projection for heads 8:12 during pairs 2-3, norm batches
                # late, PE warm-keepers in the filler-starved pairs 4-5.
                for pr in range(NP):
                    fill = {j: [] for j in range(NT)}
                    for j in range(NT):
                        t = pr * NT + j + EB_DEPTH
                        if t < NP * NT:
                            fill[j].append(
                                lambda a=t // NT, b=t % NT: eb_load(a, b))
                    nxt = pr + 1
                    if nxt < NP:
                        if pr > 0:
                            fill[1].append(lambda n=nxt: kq_sub("k", n, 0))
                            fill[2].append(lambda n=nxt: kq_sub("k", n, 1))
                        fill[4].append(lambda n=nxt: kq_sub("q", n, 0))
                        fill[5].append(lambda n=nxt: kq_sub("q", n, 1))
                    if pr == 0:
                        for nt in range(2, NT):
                            fill[nt - 2].append(lambda t=nt: v_proj(0, 8, t))
                    if pr == 2:
                        for nt in range(4):
                            fill[6].append(lambda t=nt: v_proj(8, 4, t))
                    if pr == 3:
                        for nt in range(4, NT):
                            fill[6].append(lambda t=nt: v_proj(8, 4, t))
                    if pr == 4:
                        # heads 0..7 sums are complete after pair 3 evac
                        fill[0].append(lambda: norm_batch(0))
                        for h in range(4):
                            fill[6].append(lambda hh=h: norm_mul(hh))
                        fill[3].append(pe_warm)
                        fill[7].append(pe_warm)
                    if pr == 5:
                        fill[0].append(lambda: norm_batch(1))
                        for h in range(4, 8):
                            fill[3].append(lambda hh=h: norm_mul(hh))
                        for h in range(8, 10):
                            fill[5].append(lambda hh=h: norm_mul(hh))
                        for j in (1, 3, 5, 7):
                            fill[j].append(pe_warm)
                    attn_pair(pr, fill)

                norm_batch(2)

            # ---- output projection ------------------------------------
            # [128, 512] granularity (one PSUM bank per tile, 8 in
            # flight).  Pass 1 (ci 0..4: heads 0..9, already normalized)
            # for the first 8 tiles is emitted BEFORE the heads-10/11
            # normalize-multiplies so the PE stays busy through the final
            # normalization round trip; each tile then finishes with the
            # ci=5 matmul, a small ACT bias-add, and a store on the (idle)
            # sync queue.  Emitting more than 8 pass-1 tiles would
            # deadlock the PE FIFO on pool-slot reuse.
            with tc.tile_pool(name="ops", bufs=8, space="PSUM") as pC, \
                 tc.tile_pool(name="otb", bufs=6) as otp:

                def oproj_acc(cot, nb, cis):
                    ps = pC.tile([128, F], f32, tag="o",
                                 name=f"po_{cot}_{nb}")
                    for i, ci in enumerate(cis):
                        nc.tensor.matmul(
                            ps,
                            lhsT=wpb[:, ci, cot * 128:(cot + 1) * 128],
                            rhs=atb[:, ci, nb * F:(nb + 1) * F],
                            start=(i == 0),
                            stop=False,
                        )
                    return ps

                def oproj_fin(ps, cot, nb):
                    nc.tensor.matmul(
                        ps,
                        lhsT=wpb[:, CT - 1, cot * 128:(cot + 1) * 128],
                        rhs=atb[:, CT - 1, nb * F:(nb + 1) * F],
                        start=False, stop=True,
                    )
                    ot = otp.tile([128, F], f32, tag="ot",
                                  name=f"ot_{cot}_{nb}")
                    nc.scalar.activation(
                        ot, ps, AF.Identity, bias=bpb[:, cot:cot + 1])
                    nc.sync.dma_start(
                        outT_d[cot * 128:(cot + 1) * 128,
                               nb * F:(nb + 1) * F], ot)

                pss = {}
                for cot in range(4):
                    for nb in range(2):
                        pss[(cot, nb)] = oproj_acc(cot, nb, range(CT - 1))
                norm_mul(10, nc.vector)
                norm_mul(11, nc.vector)
                # interleave: finish an early tile (frees its PSUM slot),
                # immediately start one of the two remaining column
                # tiles' accumulations in the freed slot, so the last
                # tiles don't run serially at the very end
                late = [(cot, nb) for cot in range(4, CT) for nb in range(2)]
                lps = {}
                for i, cot in enumerate(range(4)):
                    for nb in range(2):
                        oproj_fin(pss[(cot, nb)], cot, nb)
                    lps[late[i]] = oproj_acc(*late[i], range(CT - 1))
                for cot, nb in late:
                    oproj_fin(lps[(cot, nb)], cot, nb)

    nc.compile()
    return nc


def _get_nc():
    if "nc" not in _cache:
        _cache["nc"] = _build()
    return _cache["nc"]


def prep_in_maps(x, attn_bias, Wq, Wk, Wv, Wp, bp):
    """Host-side sharding + layout prep (transposes/casts/exp of bias)."""
    wqT = np.ascontiguousarray(Wq.T).astype(BF16)
    wkT = np.ascontiguousarray(Wk.T).astype(BF16)
    wvT = np.ascontiguousarray(Wv.T).astype(BF16)
    wpT = np.ascontiguousarray(Wp.T).astype(BF16)
    bpT = np.ascontiguousarray(bp.astype(np.float32).reshape(CT, 128).T)
    # exp(bias)^T packed per (pair, key-tile, query-half): see kernel docstr
    E = np.exp(attn_bias[0].astype(np.float32)).transpose(0, 2, 1)
    E = np.ascontiguousarray(E).reshape(H, NT, 128, 2, F)
    ebPk = np.empty((NP, NT, 2, 128, 2 * F), dtype=np.float32)
    for pr in range(NP):
        ebPk[pr, :, :, :, 0:F] = E[2 * pr].transpose(0, 2, 1, 3)
        ebPk[pr, :, :, :, F:2 * F] = E[2 * pr + 1].transpose(0, 2, 1, 3)
    ebPk = ebPk.astype(BF16)
    in_maps = []
    for b in range(B):
        in_maps.append({
            "xT": np.ascontiguousarray(x[b].T).astype(BF16),
            "wqT": wqT, "wkT": wkT, "wvT": wvT, "wpT": wpT,
            "bpT": bpT, "ebPk": ebPk,
        })
    return in_maps


def run(in_maps, trace=False, **kw):
    from concourse.bass_utils import run_bass_kernel_spmd

    nc = _get_nc()
    return run_bass_kernel_spmd(
        nc, in_maps, core_ids=list(range(B)), trace=trace, **kw
    )


def kernel(x, attn_bias, Wq, Wk, Wv, Wp, bp):
    res = run(prep_in_maps(x, attn_bias, Wq, Wk, Wv, Wp, bp))
    out = np.stack(
        [res.results[b]["outT"].T for b in range(B)]
    ).astype(np.float32)
    return out


# revision 45
# speedup vs baseline: 1.0224x; 1.0087x over previous
"""Multi-head attention (B=8, N=1024, C=768, H=12, D=64) on 8 TRN2 NeuronCores.

Strategy: pure data-parallel over batch (B == n_cores == 8), no collectives.
Each core computes full 12-head attention for one batch element in a fully
transposed layout (channels on SBUF partitions).

v2 design (vs. the nb-serial baseline):
  - Heads are processed in PAIRS (2i, 2i+1).  The even head's K/Q live on
    SBUF partitions 0:64, the odd head's on 64:128, so the two QK^T matmuls
    (contraction 64 each) run CONCURRENTLY in the PE array via row tiling
    (tile_position (0,0) / (64,0)) writing different PSUM banks.
  - S-pair tiles are [128, 1024] f32 (2 PSUM banks): cols 0:512 even head,
    512:1024 odd head, for one (key-tile j, query-half nb).  One FD=1024
    ACT exp per tile halves the per-instruction overhead.
  - The additive attn bias is applied as exp(S/8)*exp(bias): exp(bias) is
    precomputed on host, loaded bf16, multiplied on DVE at 2x rate in SBUF
    (the f32-PSUM add of the baseline ran at 1x and cost 131us).
  - PV keeps the ones-column trick (out rows 0:64 = unnormalized out^T,
    row 64 = softmax sum).  All four (head, nb) chains evacuate via one
    [65,512] DVE copy to SBUF, then gpsimd DMAs split rows 0:64 -> atb
    (partition-shifted for odd heads) and row 64 -> s_stage.
  - Normalization in 3 batches (heads 0-7, 8-9, 10-11) overlapped with
    attention; the last two heads' normalize-multiply runs on DVE to
    shorten the tail.  Whole norm DMA chain rides the sync queue (FIFO).
  - K/Q projection tiles for pair i+1 are interleaved into pair i's
    attention to keep the PE dense (HAM un-throttled).
"""

import os
import sys
import numpy as np

for _p in ("/opt/trn_rl_repo", "/root/.axon_site/_ro/trn_rl_repo"):
    if os.path.isdir(_p) and _p not in sys.path:
        sys.path.append(_p)

import ml_dtypes

BF16 = ml_dtypes.bfloat16

B, N, C = 8, 1024, 768
H, D = 12, 64
CT = C // 128         # 6 channel tiles
NT = N // 128         # 8 key tiles
F = 512
NP = H // 2           # 6 head pairs
HA = 8                # heads in normalization batch A (then 8-9, 10-11)

_cache = {}


def _build():
    import concourse.bass as bass
    import concourse.tile as tile
    from concourse import bacc, mybir

    f32 = mybir.dt.float32
    bf16 = mybir.dt.bfloat16
    AF = mybir.ActivationFunctionType
    ALU = mybir.AluOpType

    nc = bacc.Bacc("TRN2", target_bir_lowering=False)

    xT_d = nc.dram_tensor("xT", [C, N], bf16, kind="ExternalInput")
    wqT_d = nc.dram_tensor("wqT", [C, C], bf16, kind="ExternalInput")
    wkT_d = nc.dram_tensor("wkT", [C, C], bf16, kind="ExternalInput")
    wvT_d = nc.dram_tensor("wvT", [C, C], bf16, kind="ExternalInput")
    wpT_d = nc.dram_tensor("wpT", [C, C], bf16, kind="ExternalInput")
    bpT_d = nc.dram_tensor("bpT", [128, CT], f32, kind="ExternalInput")
    # exp(attn_bias) packed per (pair, key-tile j, query-half nb):
    # [...,0:512] = even head, [...,512:1024] = odd head
    eb_d = nc.dram_tensor("ebPk", [NP, NT, 2, 128, 2 * F], bf16,
                          kind="ExternalInput")
    outT_d = nc.dram_tensor("outT", [C, N], f32, kind="ExternalOutput")
    # softmax-sum scratch per normalization batch
    sA_scr = nc.dram_tensor("sA_scr", [HA * N], bf16)
    sB1_scr = nc.dram_tensor("sB1_scr", [2 * N], bf16)
    sB2_scr = nc.dram_tensor("sB2_scr", [2 * N], bf16)
    rA_scr = nc.dram_tensor("rA_scr", [1, HA * N], bf16)
    rB1_scr = nc.dram_tensor("rB1_scr", [1, 2 * N], bf16)
    rB2_scr = nc.dram_tensor("rB2_scr", [1, 2 * N], bf16)

    with tile.TileContext(nc) as tc:
        with tc.tile_pool(name="persist", bufs=1) as pers:
            xTb = pers.tile([128, CT, N], bf16, tag="xT")
            wqb = pers.tile([128, CT, C], bf16, tag="wq")
            wkb = pers.tile([128, CT, C], bf16, tag="wk")
            wvb = pers.tile([128, CT, C], bf16, tag="wv")
            wpb = pers.tile([128, CT, C], bf16, tag="wp")
            bpb = pers.tile([128, CT], f32, tag="bp")
            # row 64 collects softmax sums (same partition as pv row 64)
            rba = pers.tile([128, H * N], bf16, tag="rba")
            qtb = pers.tile([128, CT, N], bf16, tag="qt")
            ktb = pers.tile([128, CT, N], bf16, tag="kt")
            vb = pers.tile([128, NT, H, D + 1], bf16, tag="v")
            atb = pers.tile([128, CT, N], bf16, tag="at")

            for q0 in range(0, N, 256):
                nc.sync.dma_start(
                    xTb[:, :, q0:q0 + 256],
                    xT_d[:, q0:q0 + 256].rearrange(
                        "(ci p) n -> p ci n", p=128))
            nc.scalar.dma_start(
                wvb[:, :, 0:512],
                wvT_d[:, 0:512].rearrange("(ci p) o -> p ci o", p=128))
            nc.scalar.dma_start(
                wvb[:, :, 512:C],
                wvT_d[:, 512:C].rearrange("(ci p) o -> p ci o", p=128))
            nc.gpsimd.dma_start(
                wkb, wkT_d[:].rearrange("(ci p) o -> p ci o", p=128))
            nc.scalar.dma_start(
                wqb, wqT_d[:].rearrange("(ci p) o -> p ci o", p=128))
            nc.scalar.dma_start(bpb, bpT_d[:])

            nc.vector.memset(vb[:, :, :, D:D + 1], 1.0)

            with tc.tile_pool(name="ups", bufs=2, space="PSUM") as pU, \
                 tc.tile_pool(name="pvps", bufs=2, space="PSUM") as pPV, \
                 tc.tile_pool(name="fillps", bufs=2, space="PSUM") as pF, \
                 tc.tile_pool(name="ebb", bufs=4) as ebp, \
                 tc.tile_pool(name="vstagb", bufs=4) as vstagp, \
                 tc.tile_pool(name="nrmb", bufs=1) as nrm, \
                 tc.tile_pool(name="ptb", bufs=18) as ptp:

                eb_tiles = {}

                def eb_load(pr, j):
                    ebt = ebp.tile([128, 2, 2 * F], bf16, tag="eb")
                    nc.sync.dma_start(
                        ebt, eb_d[pr, j].rearrange("nb p q -> p nb q"))
                    eb_tiles[(pr, j)] = ebt

                def v_proj(h0, nh, nt):
                    """V projection for heads [h0, h0+nh) at key-tile nt."""
                    f0, fw = h0 * D, nh * D
                    ps = pF.tile([128, F], f32, tag="fill")
                    for ci in range(CT):
                        nc.tensor.matmul(
                            ps[:, :fw],
                            lhsT=xTb[:, ci, nt * 128:(nt + 1) * 128],
                            rhs=wvb[:, ci, f0:f0 + fw],
                            start=(ci == 0),
                            stop=(ci == CT - 1),
                        )
                    nc.vector.tensor_copy(
                        vb[:, nt, h0:h0 + nh, 0:D],
                        ps[:, :fw].rearrange("p (h d) -> p h d", d=D),
                    )

                def pe_warm():
                    """Redundant 6-matmul group (recomputes k-projection
                    tile 0 into a dead PSUM tile, never read).  Emitted in
                    filler-starved stretches so the PE's activity monitor
                    does not re-throttle the clock (K=4/8) on micro-idle.
                    Lives on the 'fill' tag, decoupled from the softmax
                    S-tile rotation."""
                    ps = pF.tile([128, F], f32, tag="fill", name="warm")
                    for ci in range(CT):
                        nc.tensor.matmul(
                            ps,
                            lhsT=wkb[:, ci, 0:128],
                            rhs=xTb[:, ci, 0:F],
                            start=(ci == 0),
                            stop=(ci == CT - 1),
                        )

                def kq_sub(which, cot, nb):
                    wb, dst = (wkb, ktb) if which == "k" else (wqb, qtb)
                    ps = pF.tile([128, F], f32, tag="fill")
                    for ci in range(CT):
                        nc.tensor.matmul(
                            ps,
                            lhsT=wb[:, ci, cot * 128:(cot + 1) * 128],
                            rhs=xTb[:, ci, nb * F:(nb + 1) * F],
                            start=(ci == 0),
                            stop=(ci == CT - 1),
                        )
                    nc.vector.tensor_copy(
                        dst[:, cot, nb * F:(nb + 1) * F], ps)

                def kq_ct(cot):
                    for which in ("k", "q"):
                        for nb in range(2):
                            kq_sub(which, cot, nb)

                def qk_pair(pr, j, nb):
                    """Row-tiled QK^T for both heads of the pair: even head
                    on PE rows 0:64, odd head on rows 64:128, concurrent."""
                    ksl = slice(j * 128, (j + 1) * 128)
                    qsl = slice(nb * F, (nb + 1) * F)
                    sp = pU.tile([128, 2 * F], f32, tag="ps",
                                 name=f"sp_{pr}_{j}_{nb}")
                    nc.tensor.matmul(
                        sp[:, 0:F],
                        lhsT=ktb[0:64, pr, ksl],
                        rhs=qtb[0:64, pr, qsl],
                        start=True, stop=True,
                    )
                    nc.tensor.matmul(
                        sp[:, F:2 * F],
                        lhsT=ktb[64:128, pr, ksl],
                        rhs=qtb[64:128, pr, qsl],
                        start=True, stop=True,
                    )
                    return sp

                def pv_evac(pr, hl, pv_nb):
                    """Evacuate one head's two PV chains; softmax-sum row
                    64 goes straight to the DRAM scratch its norm batch
                    reads."""
                    h = 2 * pr + hl
                    s_scr, hh = (
                        (sA_scr, h) if h < HA
                        else (sB1_scr, h - HA) if h < HA + 2
                        else (sB2_scr, h - HA - 2))
                    for nb in range(2):
                        qsl = slice(nb * F, (nb + 1) * F)
                        vstag = vstagp.tile([D + 1, F], bf16, tag="vstag")
                        nc.vector.tensor_copy(vstag, pv_nb[nb])
                        nc.gpsimd.dma_start(
                            atb[64 * hl:64 * hl + 64, pr, qsl],
                            vstag[0:D, :])
                        nc.gpsimd.dma_start(
                            s_scr[hh * N + nb * F:hh * N + (nb + 1) * F],
                            vstag[D:D + 1, :])

                def attn_pair(pr, fillers):
                    """Attention for heads (2*pr, 2*pr+1).

                    Emission order per j: exp/mul for j, then QK for j+1
                    (so the PE never head-of-line blocks on the softmax
                    chain), then the even head's PV matmuls for j, then
                    fillers.  The odd head's PV chains run as one dense
                    16-matmul block at the end of the pair on the pt tiles
                    buffered in SBUF - this halves the PSUM footprint of
                    PV (2 banks instead of 4), making room for the
                    decoupled 'fill' tag, and packs PE work into the pair
                    boundary.
                    """
                    pv0 = [pPV.tile([D + 1, F], f32, tag="pv",
                                    name=f"pv_{pr}_0_{nb}")
                           for nb in range(2)]
                    sps = [qk_pair(pr, 0, nb) for nb in range(2)]
                    pts_all = []
                    for j in range(NT):
                        ebt = eb_tiles.pop((pr, j))
                        pts = []
                        for nb in range(2):
                            pt = ptp.tile([128, 2 * F], bf16, tag="pt",
                                          name=f"pt_{pr}_{j}_{nb}")
                            nc.scalar.activation(
                                pt, sps[nb], AF.Exp, scale=0.125)
                            nc.vector.tensor_tensor(
                                pt, pt, ebt[:, nb, :], ALU.mult)
                            pts.append(pt)
                        pts_all.append(pts)
                        if j + 1 < NT:
                            sps = [qk_pair(pr, j + 1, nb) for nb in range(2)]
                        for nb in range(2):
                            nc.tensor.matmul(
                                pv0[nb],
                                lhsT=vb[:, j, 2 * pr, :],
                                rhs=pts[nb][:, 0:F],
                                start=(j == 0),
                                stop=(j == NT - 1),
                            )
                        for fn in fillers.get(j, ()):
                            fn()
                    pv_evac(pr, 0, pv0)
                    pv1 = [pPV.tile([D + 1, F], f32, tag="pv",
                                    name=f"pv_{pr}_1_{nb}")
                           for nb in range(2)]
                    for j in range(NT):
                        for nb in range(2):
                            nc.tensor.matmul(
                                pv1[nb],
                                lhsT=vb[:, j, 2 * pr + 1, :],
                                rhs=pts_all[j][nb][:, F:2 * F],
                                start=(j == 0),
                                stop=(j == NT - 1),
                            )
                    pv_evac(pr, 1, pv1)

                def norm_batch(batch):
                    """Batched reciprocal of softmax sums for a head range.

                    The sync-queue DMAs are FIFO-ordered among themselves;
                    the s_scr reload depends on the pair-evacuation DMAs
                    (gpsimd queue) via the DRAM tensor dependency.
                    """
                    h0, nh, s_scr, r_scr = [
                        (0, HA, sA_scr, rA_scr),
                        (HA, 2, sB1_scr, rB1_scr),
                        (HA + 2, 2, sB2_scr, rB2_scr),
                    ][batch]
                    cols = nh * N // 128
                    sb = nrm.tile([128, HA * N // 128], bf16, tag="sb")
                    nc.sync.dma_start(
                        sb[:, :cols],
                        s_scr[:].rearrange("(p f) -> p f", p=128))
                    rc32 = nrm.tile([128, HA * N // 128], f32, tag="rc32")
                    nc.vector.reciprocal(rc32[:, :cols], sb[:, :cols])
                    rcb = nrm.tile([128, HA * N // 128], bf16, tag="rcb")
                    nc.vector.tensor_copy(rcb[:, :cols], rc32[:, :cols])
                    nc.sync.dma_start(
                        r_scr[0, :].rearrange("(p f) -> p f", p=128),
                        rcb[:, :cols])
                    nc.sync.dma_start(
                        rba[:, h0 * N:(h0 + nh) * N],
                        r_scr[:].to_broadcast([128, nh * N]))

                def norm_mul(h, engine=None):
                    ct, po = h // 2, 64 * (h % 2)
                    sl = atb[po:po + 64, ct, :]
                    eng = engine or nc.gpsimd
                    eng.tensor_tensor(
                        sl, sl, rba[po:po + 64, h * N:(h + 1) * N], ALU.mult)

                # ---- emission schedule --------------------------------
                EB_DEPTH = 4
                for t in range(EB_DEPTH):
                    eb_load(0, t)
                nc.sync.dma_start(
                    wpb, wpT_d[:].rearrange("(ci p) o -> p ci o", p=128))
                v_proj(0, 8, 0)
                v_proj(0, 8, 1)
                kq_ct(0)
                kq_sub("k", 1, 0)
                kq_sub("k", 1, 1)

                # fillers per pair: rest of the V projection inside pair 0,
                # kq projection for pair+1, rolling EB prefetch, V
                # projection for heads 8:12 during pairs 2-3, norm batches
                # late, PE warm-keepers in the filler-starved pairs 4-5.
                for pr in range(NP):
                    fill = {j: [] for j in range(NT)}
                    for j in range(NT):
                        t = pr * NT + j + EB_DEPTH
                        if t < NP * NT:
                            fill[j].append(
                                lambda a=t // NT, b=t % NT: eb_load(a, b))
                    nxt = pr + 1
                    if nxt < NP:
                        if pr > 0:
                            fill[1].append(lambda n=nxt: kq_sub("k", n, 0))
                            fill[2].append(lambda n=nxt: kq_sub("k", n, 1))
                        fill[4].append(lambda n=nxt: kq_sub("q", n, 0))
                        fill[5].append(lambda n=nxt: kq_sub("q", n, 1))
                    if pr == 0:
                        for nt in range(2, NT):
                            fill[nt - 2].append(lambda t=nt: v_proj(0, 8, t))
                    if pr == 2:
                        for nt in range(4):
                            fill[6].append(lambda t=nt: v_proj(8, 4, t))
                    if pr == 3:
                        for nt in range(4, NT):
                            fill[6].append(lambda t=nt: v_proj(8, 4, t))
                    if pr == 4:
                        # heads 0..7 sums are complete after pair 3 evac
                        fill[0].append(lambda: norm_batch(0))
                        for h in range(4):
                            fill[6].append(lambda hh=h: norm_mul(hh))
                        fill[3].append(pe_warm)
                        fill[7].append(pe_warm)
                    if pr == 5:
                        fill[0].append(lambda: norm_batch(1))
                        for h in range(4, 8):
                            fill[3].append(lambda hh=h: norm_mul(hh))
                        for h in range(8, 10):
                            fill[5].append(lambda hh=h: norm_mul(hh))
                        for j in (1, 3, 5, 7):
                            fill[j].append(pe_warm)
                    attn_pair(pr, fill)

                norm_batch(2)

            # ---- output projection ------------------------------------
            # [128, 512] granularity (one PSUM bank per tile, 8 in
            # flight).  Pass 1 (ci 0..4: heads 0..9, already normalized)
            # for the first 8 tiles is emitted BEFORE the heads-10/11
            # normalize-multiplies so the PE stays busy through the final
            # normalization round trip; each tile then finishes with the
            # ci=5 matmul, a small ACT bias-add, and a store on the (idle)
            # sync queue.  Emitting more than 8 pass-1 tiles would
            # deadlock the PE FIFO on pool-slot reuse.
            with tc.tile_pool(name="ops", bufs=8, space="PSUM") as pC, \
                 tc.tile_pool(name="otb", bufs=6) as otp:

                def oproj_acc(cot, nb, cis):
                    ps = pC.tile([128, F], f32, tag="o",
                                 name=f"po_{cot}_{nb}")
                    for i, ci in enumerate(cis):
                        nc.tensor.matmul(
                            ps,
                            lhsT=wpb[:, ci, cot * 128:(cot + 1) * 128],
                            rhs=atb[:, ci, nb * F:(nb + 1) * F],
                            start=(i == 0),
                            stop=False,
                        )
                    return ps

                def oproj_fin(ps, cot, nb):
                    nc.tensor.matmul(
                        ps,
                        lhsT=wpb[:, CT - 1, cot * 128:(cot + 1) * 128],
                        rhs=atb[:, CT - 1, nb * F:(nb + 1) * F],
                        start=False, stop=True,
                    )
                    ot = otp.tile([128, F], f32, tag="ot",
                                  name=f"ot_{cot}_{nb}")
                    nc.scalar.activation(
                        ot, ps, AF.Identity, bias=bpb[:, cot:cot + 1])
                    nc.sync.dma_start(
                        outT_d[cot * 128:(cot + 1) * 128,
                               nb * F:(nb + 1) * F], ot)

                pss = {}
                for cot in range(4):
                    for nb in range(2):
                        pss[(cot, nb)] = oproj_acc(cot, nb, range(CT - 1))
                norm_mul(10, nc.vector)
                norm_mul(11, nc.vector)
                for cot in range(4):
                    for nb in range(2):
                        oproj_fin(pss[(cot, nb)], cot, nb)
                for cot in range(4, CT):
                    for nb in range(2):
                        ps = oproj_acc(cot, nb, range(CT - 1))
                        oproj_fin(ps, cot, nb)

    nc.compile()
    return nc


def _get_nc():
    if "nc" not in _cache:
        _cache["nc"] = _build()
    return _cache["nc"]


def prep_in_maps(x, attn_bias, Wq, Wk, Wv, Wp, bp):
    """Host-side sharding + layout prep (transposes/casts/exp of bias)."""
    wqT = np.ascontiguousarray(Wq.T).astype(BF16)
    wkT = np.ascontiguousarray(Wk.T).astype(BF16)
    wvT = np.ascontiguousarray(Wv.T).astype(BF16)
    wpT = np.ascontiguousarray(Wp.T).astype(BF16)
    bpT = np.ascontiguousarray(bp.astype(np.float32).reshape(CT, 128).T)
    # exp(bias)^T packed per (pair, key-tile, query-half): see kernel docstr
    E = np.exp(attn_bias[0].astype(np.float32)).transpose(0, 2, 1)
    E = np.ascontiguousarray(E).reshape(H, NT, 128, 2, F)
    ebPk = np.empty((NP, NT, 2, 128, 2 * F), dtype=np.float32)
    for pr in range(NP):
        ebPk[pr, :, :, :, 0:F] = E[2 * pr].transpose(0, 2, 1, 3)
        ebPk[pr, :, :, :, F:2 * F] = E[2 * pr + 1].transpose(0, 2, 1, 3)
    ebPk = ebPk.astype(BF16)
    in_maps = []
    for b in range(B):
        in_maps.append({
            "xT": np.ascontiguousarray(x[b].T).astype(BF16),
            "wqT": wqT, "wkT": wkT, "wvT": wvT, "wpT": wpT,
            "bpT": bpT, "ebPk": ebPk,
        })
    return in_maps


def run(in_maps, trace=False, **kw):
    from concourse.bass_utils import run_bass_kernel_spmd

    nc = _get_nc()
    return run_bass_kernel_spmd(
        nc, in_maps, core_ids=list(range(B)), trace=trace, **kw
    )


def kernel(x, attn_bias, Wq, Wk, Wv, Wp, bp):
    res = run(prep_in_maps(x, attn_bias, Wq, Wk, Wv, Wp, bp))
    out = np.stack(
        [res.results[b]["outT"].T for b in range(B)]
    ).astype(np.float32)
    return out
